# revision 17
# baseline (speedup 1.0000x reference)
"""Trainium2 Bass kernel for nn_DecGreenNet_product_CP3.

Reference computation:
    lhs  = tanh(input @ Wx1 + bx1) @ Wx2 + bx2          # [B, 512]
    s_i  = sum_n sin(pi*eq*qx_n) * mlp_i(qx_n)           # [8,16] per branch
    rhs  = einsum('bx,dx,fx->bdf', s_a, s_c, s_e)        # [512]
    out  = lhs @ rhs                                     # [B]

Restructurings used here:
  1) out[b] = tanh(input[b] @ Wx1 + bx1) @ (Wx2 @ rhs) + bx2 @ rhs
     collapses the [B,512]x[512,512] GEMM into a matvec.
  2) The quad branch is a 1-D quadrature: z[h] = sum_n y_n tanh(w_h qx_n + b_h)
     with y = sin(pi*eq*qx).  Fit tanh(w_h x + b_h) on x in [0,1] with a
     degree-D polynomial in u = 2x-1 (coefficients from a G-point Chebyshev
     grid via a host-precomputed pseudoinverse), so
         z[h] ~= sum_j c_j[h] * m_j,   m_j = sum_n y_n u_n^j .
     The moments m_j are tiny DVE work; no per-node MLP is needed, so every
     core computes the full quadrature locally and NO collective is needed
     (validated to ~1e-3 final rel err in fp32 simulation).

Sharding: batch B split 8 ways (8192 rows/core); quadrature replicated.

Main loop: L1 hidden chunks ([128h,512b] x4 h-tiles, row-tiled into the PE
array at partitions 0/32/64/96 so the 4 matmuls run concurrently), tanh on
ScalarE (the bottleneck engine, kept saturated), dot chunks interleaved in
PE program order, per-pair output scaling on GpSimd, fp16 matvec with
2^-36/2^36 scaling as in the validated baseline numerics.
"""

import numpy as np

import concourse.bacc as bacc
import concourse.bass as bass
import concourse.mybir as mybir
import concourse.tile as tile
from concourse.bass_utils import run_bass_kernel_spmd

F32 = mybir.dt.float32
F16 = mybir.dt.float16
AF = mybir.ActivationFunctionType
ALU = mybir.AluOpType
AX = mybir.AxisListType

NCORES = 8
B, DIN, H = 65536, 3, 512
N, HQ = 8192, 128
S0, RX = 8, 16
BL = B // NCORES          # 8192 batch rows per core
CH = 512                  # batch chunk (columns per matmul)
NCH = BL // CH            # 16 chunks
NT = N // 128             # 64 node columns (full quadrature per core)

PDEG = 10                 # poly degree in u = 2*qx-1
NC_ = PDEG + 1            # coefficients per branch
G = 64                    # tanh fit grid size

# fp16 scaling for the matvec chain (w ~ 1e10..1e11)
RC_SCALE = 2.0 ** -36
OUT_SCALE = 2.0 ** 36

# minimax odd polynomial for sin(t), t in [0, pi]: sin(t)=t*P(t^2), err<2e-5
SIN_C = (0.999984590176674, -0.16663258473611252, 8.312385898666645e-03,
         -1.9316230946716391e-04, 2.1732361127812407e-06)

DOT_LAG = 8               # dot(c-DOT_LAG) emitted after L1(c)

_CACHED_NC = None

import os
_STAGE = os.environ.get("K_STAGE", "full")   # m | s | eins | full


def _host_grid_P():
    """Chebyshev grid on [0,1] and pinv mapping grid samples -> monomial
    coefficients in u = 2x-1 (host float64, cast to fp32)."""
    g = np.cos((2 * np.arange(G) + 1) / (2 * G) * np.pi)     # (-1,1)
    xg = (g + 1.0) / 2.0                                     # (0,1)
    u = 2.0 * xg - 1.0
    V = np.stack([u ** j for j in range(NC_)], axis=1)       # [G, NC_]
    P = np.linalg.pinv(V)                                    # [NC_, G]
    return xg, P


_XG, _P = _host_grid_P()


def _build():
    nc = bacc.Bacc("TRN2", target_bir_lowering=False, debug=False,
                   num_devices=NCORES)

    xT = nc.dram_tensor("xT", [DIN + 1, BL], F16, kind="ExternalInput").ap()
    wx1t = nc.dram_tensor("wx1t", [128, 128], F16, kind="ExternalInput").ap()
    wx2t = nc.dram_tensor("wx2tb", [64, 4096], F16, kind="ExternalInput").ap()
    bx2r = nc.dram_tensor("bx2rb", [64, 128], F16, kind="ExternalInput").ap()
    qxc = nc.dram_tensor("qxc", [128, 3 * NT], F32, kind="ExternalInput").ap()
    wqa = nc.dram_tensor("wqa", [6, HQ], F16, kind="ExternalInput").ap()
    wq2 = nc.dram_tensor("wq2", [HQ, 3 * HQ], F32, kind="ExternalInput").ap()
    bq2r = nc.dram_tensor("bq2r", [3, HQ], F32, kind="ExternalInput").ap()
    eqb = nc.dram_tensor("eqb", [128, 1], F32, kind="ExternalInput").ap()
    gx2 = nc.dram_tensor("gx2", [2, G], F16, kind="ExternalInput").ap()
    ptm = nc.dram_tensor("ptm", [G, NC_], F32, kind="ExternalInput").ap()
    out_d = nc.dram_tensor("out", [BL], F32, kind="ExternalOutput").ap()

    global _APS
    _APS = (xT, wx1t, wx2t, bx2r, qxc, wqa, wq2, bq2r, eqb, gx2, ptm, out_d)
    with tile.TileContext(nc) as tc:
        _body(nc, tc)
    nc.compile()
    return nc


def _body(nc, tc):
    xT, wx1t, wx2t, bx2r, qxc, wqa, wq2, bq2r, eqb, gx2, ptm, out_d = _APS
    with (
        tc.tile_pool(name="const", bufs=1) as constp,
        tc.tile_pool(name="qsb", bufs=1) as qsb,
        tc.tile_pool(name="dram", bufs=1, space="DRAM") as dram,
        tc.tile_pool(name="mainsb", bufs=1) as mainsb,
        tc.tile_pool(name="hidp", bufs=10) as hidp,
        tc.tile_pool(name="orowp", bufs=2) as orowp,
        tc.tile_pool(name="prep", bufs=2, space="PSUM") as prep,
        tc.tile_pool(name="quadp", bufs=2, space="PSUM") as quadp,
        tc.tile_pool(name="outp", bufs=1, space="PSUM") as outpp,
    ):
        ones128 = constp.tile([128, 1], F32)
        nc.vector.memset(ones128, 1.0)

        # ---------------- DMAs: L1 + quad-chain inputs first -------------
        qxc_sb = qsb.tile([128, 3 * NT], F32, tag="qxc")
        nc.sync.dma_start(out=qxc_sb, in_=qxc)
        wx1t_sb = mainsb.tile([128, 128], F16, tag="wx1t")
        nc.gpsimd.dma_start(out=wx1t_sb, in_=wx1t)
        xT4_sb = mainsb.tile([128, BL], F16, tag="xT4")
        for i in range(2):
            nc.sync.dma_start(out=xT4_sb[32 * i:32 * i + 4, :], in_=xT)
        for i in range(2, 4):
            nc.gpsimd.dma_start(out=xT4_sb[32 * i:32 * i + 4, :], in_=xT)
        eqb_sb = qsb.tile([128, 1], F32, tag="eqb")
        nc.sync.dma_start(out=eqb_sb, in_=eqb)
        gx2_sb = qsb.tile([2, G], F16, tag="gx2")
        nc.gpsimd.dma_start(out=gx2_sb, in_=gx2)
        wqa_sb = []
        for br in range(3):
            t = qsb.tile([2, HQ], F16, tag=f"wqa{br}")
            nc.gpsimd.dma_start(out=t, in_=wqa[2 * br:2 * br + 2, :])
            wqa_sb.append(t)
        ptm_sb = qsb.tile([G, NC_], F32, tag="ptm")
        nc.gpsimd.dma_start(out=ptm_sb, in_=ptm)

        # quad layer-2 + einsum weights (needed ~20us on)
        wq2_sb = qsb.tile([HQ, 3 * HQ], F32, tag="wq2")
        nc.sync.dma_start(out=wq2_sb, in_=wq2)
        bq2r_sb = []
        for br in range(3):
            t = qsb.tile([1, HQ], F32, tag=f"bq2r{br}")
            nc.sync.dma_start(out=t, in_=bq2r[br:br + 1, :])
            bq2r_sb.append(t)
        wx2t_sb = mainsb.tile([64, 4096], F16, tag="wx2t")
        nc.gpsimd.dma_start(out=wx2t_sb, in_=wx2t)
        bx2r_sb = mainsb.tile([64, 128], F16, tag="bx2r")
        nc.gpsimd.dma_start(out=bx2r_sb, in_=bx2r)

        # ---------------- quad: y = sin(pi*eq*qx), moments on DVE --------
        eqpi = qsb.tile([128, 1], F32, tag="eqpi")
        nc.vector.tensor_scalar_mul(eqpi, eqb_sb, float(np.pi))
        tq = qsb.tile([128, 3 * NT], F32, tag="tq")
        nc.vector.tensor_scalar_mul(tq, qxc_sb, eqpi[:, 0:1])
        t2 = qsb.tile([128, 3 * NT], F32, tag="t2")
        nc.vector.tensor_tensor(out=t2, in0=tq, in1=tq, op=ALU.mult)
        pp = qsb.tile([128, 3 * NT], F32, tag="pp")
        c1, c3, c5, c7, c9 = [float(v) for v in SIN_C]
        nc.vector.tensor_scalar(out=pp, in0=t2, scalar1=c9, scalar2=c7,
                                op0=ALU.mult, op1=ALU.add)
        for cof in (c5, c3, c1):
            nc.vector.tensor_tensor(out=pp, in0=pp, in1=t2, op=ALU.mult)
            nc.vector.tensor_scalar_add(pp, pp, cof)
        # u = 2*qx - 1
        u_sb = qsb.tile([128, 3 * NT], F32, tag="usb")
        nc.vector.tensor_scalar(out=u_sb, in0=qxc_sb, scalar1=2.0,
                                scalar2=-1.0, op0=ALU.mult, op1=ALU.add)
        # stack[:, j, br, t] = y * u^j  (chain); y into j=0 slot
        stack = qsb.tile([128, NC_ * 3 * NT], F32, tag="stack")
        stk = stack.rearrange("p (j c t) -> p j c t", j=NC_, t=NT)
        nc.vector.tensor_tensor(out=stk[:, 0, :, :],
                                in0=pp, in1=tq, op=ALU.mult)
        u3 = u_sb.rearrange("p (c t) -> p c t", t=NT)
        for j in range(1, NC_):
            nc.vector.tensor_tensor(out=stk[:, j, :, :],
                                    in0=stk[:, j - 1, :, :], in1=u3,
                                    op=ALU.mult)
        red = qsb.tile([128, NC_ * 3], F32, tag="red")
        nc.vector.tensor_reduce(out=red, in_=stk, axis=AX.X, op=ALU.add)
        red3 = red.rearrange("p (j c) -> p j c", c=3)

        # ---------------- main L1 chunks (row-tiled 4-pack) --------------
        hid_tiles = {}

        def emit_l1(c):
            hid = hidp.tile([128, 4 * CH], F16, tag="hid")
            for half in range(2):
                pre = prep.tile([128, 2 * CH], F32, tag="pre")
                for k in range(2):
                    ht = half * 2 + k
                    bp = 32 * ht
                    nc.tensor.matmul(
                        pre[:, k * CH:(k + 1) * CH],
                        lhsT=wx1t_sb[bp:bp + 4, :],
                        rhs=xT4_sb[bp:bp + 4, c * CH:(c + 1) * CH],
                        start=True, stop=True,
                        tile_position=(bp, 0))
                nc.scalar.activation(
                    out=hid[:, half * 2 * CH:(half + 1) * 2 * CH],
                    in_=pre, func=AF.Tanh)
            hid_tiles[c] = hid

        # HAM warmup: dense dummy matmuls while the PE would otherwise idle
        # (keeps the PE clock at 2.4 GHz instead of the cold 1.2 GHz)
        dummy_ps = outpp.tile([1, 2 * CH], F32, tag="op")

        def emit_warm(n):
            for _ in range(n):
                nc.tensor.matmul(dummy_ps[0:1, 0:CH],
                                 lhsT=xT4_sb[0:4, 0:1],
                                 rhs=xT4_sb[0:4, 0:CH],
                                 start=True, stop=True)

        emit_warm(8)
        emit_l1(0)

        # quad: tanh grid eval (tiny; feeds the c = P @ t matmuls)
        tg_sb = []
        for br in range(3):
            tg_ps = quadp.tile([G, HQ], F32, tag="qp")
            nc.tensor.matmul(tg_ps, lhsT=gx2_sb, rhs=wqa_sb[br],
                             start=True, stop=True)
            t_sb = qsb.tile([G, HQ], F32, tag=f"tsb{br}")
            nc.scalar.activation(out=t_sb, in_=tg_ps, func=AF.Tanh)
            tg_sb.append(t_sb)

        emit_l1(1)
        emit_warm(2)
        emit_l1(2)
        emit_warm(2)

        # ---------------- quad: m -> c -> z -> s (tiny MMs) --------------
        m_ps = quadp.tile([NC_, 3], F32, tag="qp")
        for br in range(3):
            nc.tensor.matmul(m_ps[:, br:br + 1], lhsT=red3[:, :, br],
                             rhs=ones128, start=True, stop=True)
        m_sb = qsb.tile([NC_, 3], F32, tag="msb")
        nc.vector.tensor_copy(out=m_sb, in_=m_ps)
        if _STAGE == "m":
            nc.sync.dma_start(out=out_d[0:NC_ * 3]
                              .rearrange("(p c) -> p c", c=3), in_=m_sb)
            return

        c_sb = []
        for br in range(3):
            c_ps = quadp.tile([NC_, HQ], F32, tag="qp")
            nc.tensor.matmul(c_ps, lhsT=ptm_sb, rhs=tg_sb[br],
                             start=True, stop=True)
            cs = qsb.tile([NC_, HQ], F32, tag=f"csb{br}")
            nc.vector.tensor_copy(out=cs, in_=c_ps)
            c_sb.append(cs)
        emit_l1(3)
        emit_warm(2)
        z_ps = quadp.tile([128, 3], F32, tag="qp")
        for br in range(3):
            nc.tensor.matmul(z_ps[:, br:br + 1], lhsT=c_sb[br],
                             rhs=m_sb[:, br:br + 1],
                             start=True, stop=True)
        z_sb = qsb.tile([128, 3], F32, tag="zsb")
        nc.vector.tensor_copy(out=z_sb, in_=z_ps)
        s_ps = quadp.tile([128, 3], F32, tag="qp")
        for br in range(3):
            nc.tensor.matmul(s_ps[:, br:br + 1],
                             lhsT=wq2_sb[:, br * HQ:(br + 1) * HQ],
                             rhs=z_sb[:, br:br + 1], start=True, stop=False)
            nc.tensor.matmul(s_ps[:, br:br + 1],
                             lhsT=bq2r_sb[br],
                             rhs=m_sb[0:1, br:br + 1],
                             start=False, stop=True)
        s_sb = qsb.tile([128, 3], F32, tag="ssb")
        nc.vector.tensor_copy(out=s_sb, in_=s_ps)
        if _STAGE == "s":
            nc.sync.dma_start(out=out_d[0:384],
                              in_=s_sb.rearrange("p c -> (p c)"))
            return

        # transpose bounce: s[(b,x), br] -> sT[x, (br, b)] via DRAM
        bounce = dram.tile([16, 24], F32, tag="bounce")
        nc.gpsimd.dma_start(out=bounce.rearrange("x (c b) -> b x c", b=8),
                            in_=s_sb)

        emit_l1(4)

        # ---------------- einsum + w chain ----------------
        sT_sb = qsb.tile([16, 24], F32, tag="sT")
        nc.sync.dma_start(out=sT_sb, in_=bounce)
        sc_ap = sT_sb[:, 8:16]
        se_ap = sT_sb[:, 16:24]
        in0 = bass.AP(tensor=sc_ap.tensor, offset=sc_ap.offset,
                      ap=[sc_ap.ap[0], sc_ap.ap[1], [0, 8]])
        in1 = bass.AP(tensor=se_ap.tensor, offset=se_ap.offset,
                      ap=[se_ap.ap[0], [0, 8], se_ap.ap[1]])
        E_sb = qsb.tile([16, 64], F32, tag="E")
        nc.vector.tensor_tensor(
            out=E_sb.rearrange("p (d f) -> p d f", f=8),
            in0=in0, in1=in1, op=ALU.mult)
        rhsp = quadp.tile([64, 8], F32, tag="qp")
        nc.tensor.matmul(rhsp, lhsT=E_sb, rhs=sT_sb[:, 0:8],
                         start=True, stop=True)
        r16 = qsb.tile([64, 8], F16, tag="r16")
        nc.vector.tensor_scalar_mul(r16, rhsp, float(RC_SCALE))
        if _STAGE == "eins":
            nc.sync.dma_start(
                out=out_d[0:512].rearrange("(p c) -> p c", c=8), in_=r16)
            return

        emit_l1(5)

        # w = Wx2 @ rhs_vec as [128, 4] fp16 (scaled by 2^-36)
        wps = quadp.tile([128, 4], F32, tag="qp")

        def emit_wps(it):
            for b in range(8):
                nc.tensor.matmul(
                    wps[:, it:it + 1],
                    lhsT=wx2t_sb[:, b * 512 + it * 128:b * 512 + (it + 1) * 128],
                    rhs=r16[:, b:b + 1],
                    start=(b == 0), stop=(b == 7))

        emit_wps(0)
        emit_wps(1)
        emit_l1(6)
        emit_wps(2)
        emit_wps(3)
        emit_l1(7)
        w_sb = qsb.tile([128, 4], F16, tag="wsb")
        nc.vector.tensor_copy(out=w_sb, in_=wps)
        # c scalar (scaled), replicated over 16 partitions; use [0,0]
        c16p = quadp.tile([16, 1], F32, tag="qp")
        for b in range(8):
            nc.tensor.matmul(
                c16p, lhsT=bx2r_sb[:, b * 16:(b + 1) * 16],
                rhs=r16[:, b:b + 1],
                start=(b == 0), stop=(b == 7))
        c16_sb = qsb.tile([16, 1], F32, tag="c16")
        nc.vector.tensor_copy(out=c16_sb, in_=c16p)
        emit_l1(8)

        # ---------------- dots interleaved with remaining L1 -------------
        pair_ps = {}

        def emit_dot(c):
            p, off = c // 2, (c % 2) * CH
            if c % 2 == 0:
                pair_ps[p] = outpp.tile([1, 2 * CH], F32, tag="op",
                                        name=f"op{p}")
            op = pair_ps[p]
            for ht in range(4):
                nc.tensor.matmul(
                    op[0:1, off:off + CH],
                    lhsT=w_sb[:, ht:ht + 1],
                    rhs=hid_tiles[c][:, ht * CH:(ht + 1) * CH],
                    start=(ht == 0), stop=(ht == 3))
            if c % 2 == 1:
                orow = orowp.tile([1, 2 * CH], F32, tag="orow")
                nc.vector.tensor_scalar(
                    out=orow, in0=op, scalar1=c16_sb[0:1, 0:1],
                    scalar2=float(OUT_SCALE), op0=ALU.add, op1=ALU.mult)
                nc.sync.dma_start(
                    out=out_d[(p * 2) * CH:(p * 2 + 2) * CH]
                        .rearrange("(o b) -> o b", o=1),
                    in_=orow)

        for c in range(9, NCH):
            emit_l1(c)
            emit_dot(c - 9)
        for c in range(NCH - 9, NCH):
            emit_dot(c)


def _get_nc():
    global _CACHED_NC
    if _CACHED_NC is None:
        _CACHED_NC = _build()
    return _CACHED_NC


def _prep_in_maps(inputs):
    f = lambda k: np.ascontiguousarray(np.asarray(inputs[k], np.float32))
    inputx = f("input")
    eq = float(np.asarray(inputs["eq_param"]).reshape(-1)[0])
    Wx1, bx1 = f("Wx1"), f("bx1")
    Wx2, bx2 = f("Wx2"), f("bx2")

    # wx1t: row-tiled L1 weights: partition 32i+k = (Wx1 row k | bx1)[h-tile i]
    wx1t = np.zeros((128, 128), np.float16)
    wx1a = np.concatenate([Wx1, bx1[None, :]], axis=0)       # [4, 512]
    for i in range(4):
        wx1t[32 * i:32 * i + 4, :] = wx1a[:, i * 128:(i + 1) * 128]
    # wx2tb[df, b*512+it*128+i] = Wx2T[b*64+df, it*128+i]
    wx2tb = np.ascontiguousarray(
        Wx2.T.reshape(8, 64, 4, 128).transpose(1, 0, 2, 3).reshape(64, 4096)
    ).astype(np.float16)
    # bx2rb[df, b*16+m] = bx2[b*64+df]
    bx2rb = np.ascontiguousarray(
        np.repeat(bx2.reshape(8, 64).T[:, :, None], 16, axis=2).reshape(64, 128)
    ).astype(np.float16)
    wqa = np.empty((6, HQ), np.float16)
    bq2r = np.empty((3, HQ), np.float32)
    wq2 = np.empty((HQ, 3 * HQ), np.float32)
    qxc = np.empty((128, 3 * NT), np.float32)
    for br, (qk, w1k, b1k, w2k, b2k) in enumerate([
            ("quad_x0", "Wq01", "bq01", "Wq02", "bq02"),
            ("quad_x1", "Wq11", "bq11", "Wq12", "bq12"),
            ("quad_x2", "Wq21", "bq21", "Wq22", "bq22")]):
        wqa[2 * br] = f(w1k)[0]
        wqa[2 * br + 1] = f(b1k)
        wq2[:, br * HQ:(br + 1) * HQ] = f(w2k)
        bq2r[br] = f(b2k)
        qxc[:, br * NT:(br + 1) * NT] = f(qk)[:, 0].reshape(NT, 128).T
    eqb = np.full((128, 1), eq, np.float32)
    gx2 = np.stack([_XG, np.ones(G)], axis=0).astype(np.float16)  # [2, G]
    ptm = np.ascontiguousarray(_P.T).astype(np.float32)           # [G, NC_]

    shared = dict(wx1t=wx1t, wx2tb=wx2tb, bx2rb=bx2rb, wqa=wqa, wq2=wq2,
                  bq2r=bq2r, eqb=eqb, qxc=qxc, gx2=gx2, ptm=ptm)
    in_maps = []
    ones_row = np.ones((1, BL), np.float32)
    for c in range(NCORES):
        ish = inputx[c * BL:(c + 1) * BL]                        # [8192, 3]
        xTm = np.concatenate([ish.T, ones_row], axis=0)          # [4, 8192]
        m = dict(shared)
        m["xT"] = np.ascontiguousarray(xTm).astype(np.float16)
        in_maps.append(m)
    return in_maps


def _run(inputs, **kw):
    nc = _get_nc()
    in_maps = _prep_in_maps(inputs)
    res = run_bass_kernel_spmd(nc, in_maps, list(range(NCORES)), **kw)
    out = np.concatenate([res.results[c]["out"].reshape(-1)
                          for c in range(NCORES)]).astype(np.float32)
    return out, res


def kernel(**inputs) -> np.ndarray:
    out, _ = _run(inputs)
    return out


def kernel_traced(**inputs):
    """Correctness + NTFF profile (exec_time_ns) in one run."""
    return _run(inputs, trace=True)
